# revision 6
# baseline (speedup 1.0000x reference)
"""Trainium2 Bass kernel for nn_ContextProjector (moe_routing).

Reference computation:
    projected = split_heads(x @ W_x + b_x)            # (B,H,N,D)
    fx        = split_heads(x @ W_fx + b_fx)          # (B,H,N,D)
    sp        = projected @ W_slice + b_slice         # (B,H,N,S)
    w         = softmax(sp / clip(temp,.5,5))         # (B,H,N,S)
    norm      = w.sum(axis=N)                         # (B,H,S)
    out       = einsum('bhns,bhnd->bhsd', w/(norm+.01), fx)

Key algebraic restructuring (all exact):
  * projected is only used for sp, so fold on host:
        Wc[c,(h,s)] = sum_d W_x[c,(h,d)] W_slice[d,s] / t[h]
        bc[(h,s)]   = (b_x[h] @ W_slice + b_slice) / t[h]
    and sp/t = x @ Wc + bc.
  * b_fx folds out of the device loop:
        sum_n w~[n,s] (fx0[n,d]+b_fx[d]) = Q[s,d] + M[s] b_fx[d]
    where Q uses bias-free fx0 and M = sum_n w~ (computed anyway).
  * The final divide by (norm+0.01) happens on host.

Device per core (8 cores: core = 4*b + quarter-of-N, 16384 tokens each):
  logits = xT.T @ Wc (+bias via a constant ones-row K-chunk), fx0 = xT.T @ W_fx,
  w~ = rowwise softmax over S=64 groups, then PSUM-accumulated reduction
  matmuls per head-pair: out[128,130] += [w_h,w_h']^T @ [fx_h|1, fx_h'|1].
  Host sums partials over the 4 N-shards and unscrambles the block layout.

All matmul operands are fp16 (PSUM accumulates fp32); everything else fp32.
"""

import numpy as np

import concourse.bass as bass
import concourse.mybir as mybir
import concourse.tile as tile
from concourse import bacc
from concourse.bass_utils import run_bass_kernel_spmd

# Problem shape (hardcoded per contract)
B, N, C = 2, 65536, 256
H, D, S = 8, 64, 64
HS = H * S    # 512
HD = H * D    # 512
P = 128
NCORES = 8
SHARDS_PER_B = NCORES // B   # 4
T = N // SHARDS_PER_B        # 16384 tokens per core
DA = D + 1                   # fx augmented with a ones column -> norm

f16 = mybir.dt.float16
f32 = mybir.dt.float32


def _emit(ctx, tc, xt, wc, wf, out, t_tokens, tt):
    nc = tc.nc
    KO = C // P              # 2 K-chunks of x
    n_blk = t_tokens // tt
    n_sub = tt // P

    consts = ctx.enter_context(tc.tile_pool(name="consts", bufs=1))
    xpool = ctx.enter_context(tc.tile_pool(name="xpool", bufs=3))
    wpool = ctx.enter_context(tc.tile_pool(name="wpool", bufs=3))
    fpool = ctx.enter_context(tc.tile_pool(name="fpool", bufs=3))
    spool = ctx.enter_context(tc.tile_pool(name="spool", bufs=3))
    ppool = ctx.enter_context(tc.tile_pool(name="ppool", bufs=2, space="PSUM"))
    apool = ctx.enter_context(tc.tile_pool(name="apool", bufs=1, space="PSUM"))
    opool = ctx.enter_context(tc.tile_pool(name="opool", bufs=1))

    # Constant weights, resident in SBUF for the whole kernel.
    wc_sb = consts.tile([P, KO + 1, HS], f16)
    nc.sync.dma_start(wc_sb[:], wc[:].rearrange("(ko ki) n -> ki ko n", ki=P))
    wf_sb = consts.tile([P, KO, HD], f16)
    nc.sync.dma_start(wf_sb[:], wf[:].rearrange("(ko ki) n -> ki ko n", ki=P))
    # Bias K-chunk lhsT: row 0 ones, rest zero -> adds wc row C (= bc) once.
    xpad = consts.tile([P, P], f16)
    nc.vector.memset(xpad[:], 0.0)
    nc.vector.memset(xpad[0:1, :], 1.0)

    # Persistent PSUM accumulators: 4 head-pair blocks of [128, 130], one
    # PSUM bank each. Pair j: head 2j result at rows 0:64 cols 0:65,
    # head 2j+1 at rows 64:128 cols 65:130 (the other two blocks are garbage).
    accs = [apool.tile([P, 130], f32, tag=f"acc{j}", name=f"acc{j}")
            for j in range(4)]

    xt_r = xt[:].rearrange("(ko ki) t -> ki ko t", ki=P)

    for blk in range(n_blk):
        x_sb = xpool.tile([P, KO, tt], f16)
        nc.sync.dma_start(x_sb[:], xt_r[:, :, blk * tt:(blk + 1) * tt])
        for sub in range(n_sub):
            gi = blk * n_sub + sub
            first = gi == 0
            last = gi == n_blk * n_sub - 1
            lg = ppool.tile([P, HS], f32, tag="lg")
            fx = ppool.tile([P, HD], f32, tag="fx")
            xk0 = x_sb[:, 0, sub * P:(sub + 1) * P]
            xk1 = x_sb[:, 1, sub * P:(sub + 1) * P]
            # logits finish on the 4th matmul so ACT can start while PE
            # still runs fx's last chunk.
            nc.tensor.matmul(lg[:], xpad[:], wc_sb[:, KO], start=True, stop=False)
            nc.tensor.matmul(lg[:], xk0, wc_sb[:, 0], start=False, stop=False)
            nc.tensor.matmul(fx[:], xk0, wf_sb[:, 0], start=True, stop=False)
            nc.tensor.matmul(lg[:], xk1, wc_sb[:, 1], start=False, stop=True)
            nc.tensor.matmul(fx[:], xk1, wf_sb[:, 1], start=False, stop=True)

            w = wpool.tile([P, HS], f16)
            w3 = w[:].rearrange("p (h s) -> p h s", h=H)
            nc.scalar.activation(out=w[:], in_=lg[:],
                                 func=mybir.ActivationFunctionType.Exp)
            den = spool.tile([P, H], f32, tag="den")
            nc.vector.tensor_reduce(out=den[:], in_=w3,
                                    axis=mybir.AxisListType.X,
                                    op=mybir.AluOpType.add)
            rec = spool.tile([P, H], f32, tag="rec")
            nc.vector.reciprocal(rec[:], den[:])
            nc.vector.tensor_mul(out=w3, in0=w3,
                                 in1=rec[:, :, None].to_broadcast((P, H, S)))

            fxa = fpool.tile([P, H, DA], f16)
            nc.scalar.activation(out=fxa[:, :, 0:D],
                                 in_=fx[:].rearrange("p (h d) -> p h d", h=H),
                                 func=mybir.ActivationFunctionType.Copy)
            nc.gpsimd.memset(fxa[:, :, D:DA], 1.0)

            for j in range(4):
                lhsT = w[:, j * P:(j + 1) * P]          # [128(tok), 128] two heads
                rhs = fxa[:, 2 * j:2 * j + 2, :]        # [128(tok), 2, 65]
                nc.tensor.matmul(accs[j][:], lhsT, rhs, start=first, stop=last)

    out_sb = opool.tile([P, 520], f32)
    for j in range(4):
        nc.vector.tensor_copy(out_sb[:, j * 130:(j + 1) * 130], accs[j][:])
    nc.sync.dma_start(out[:], out_sb[:])


def build_bass(t_tokens=T, tt=2048, finalize=True):
    from contextlib import ExitStack
    nc = bacc.Bacc("TRN2")
    xt = nc.dram_tensor("xt", [C, t_tokens], f16, kind="ExternalInput")
    wc = nc.dram_tensor("wc", [C + P, HS], f16, kind="ExternalInput")
    wf = nc.dram_tensor("wf", [C, HD], f16, kind="ExternalInput")
    out = nc.dram_tensor("out", [P, 520], f32, kind="ExternalOutput")
    with tile.TileContext(nc) as tc:
        with ExitStack() as ctx:
            _emit(ctx, tc, xt, wc, wf, out, t_tokens, tt)
    if finalize:
        nc.finalize()
    return nc


def make_device_weights(W_x, b_x, W_fx, W_slice, b_slice, temperature):
    """Host-side weight fusion -> (wc_dev [C+128, HS] f16, wf_dev [C, HD] f16)."""
    temp = np.clip(np.asarray(temperature, np.float64).reshape(H), 0.5, 5.0)
    Wx3 = np.asarray(W_x, np.float64).reshape(C, H, D)
    Ws = np.asarray(W_slice, np.float64)
    Wc = np.einsum("chd,ds->chs", Wx3, Ws) / temp[None, :, None]
    bc = (np.asarray(b_x, np.float64).reshape(H, D) @ Ws
          + np.asarray(b_slice, np.float64)[None, :]) / temp[:, None]
    wc_dev = np.zeros((C + P, HS), np.float16)
    wc_dev[:C] = Wc.reshape(C, HS).astype(np.float16)
    wc_dev[C] = bc.reshape(HS).astype(np.float16)
    wf_dev = np.asarray(W_fx).astype(np.float16)
    return wc_dev, wf_dev


def untangle(M):
    """Per-core device output [128, 520] -> Q [H, S, D+1] (col D = norm)."""
    M = np.asarray(M, np.float64)
    Q = np.empty((H, S, DA), np.float64)
    for j in range(4):
        blk = M[:, j * 130:(j + 1) * 130]
        Q[2 * j] = blk[0:S, 0:DA]
        Q[2 * j + 1] = blk[S:2 * S, DA:2 * DA]
    return Q


def postprocess(core_outs, b_fx):
    bfx = np.asarray(b_fx, np.float64).reshape(H, D)
    out = np.empty((B, H, S, D), np.float32)
    for b in range(B):
        Q = sum(untangle(core_outs[b * SHARDS_PER_B + q]) for q in range(SHARDS_PER_B))
        Mn = Q[..., D]                      # [H, S] total softmax mass
        res = (Q[..., :D] + Mn[..., None] * bfx[:, None, :]) / (Mn[..., None] + 0.01)
        out[b] = res.astype(np.float32)
    return out


def make_in_maps(x, wc_dev, wf_dev):
    x = np.asarray(x)
    in_maps = []
    for core in range(NCORES):
        b, q = core // SHARDS_PER_B, core % SHARDS_PER_B
        xt = np.ascontiguousarray(x[b, q * T:(q + 1) * T, :].T.astype(np.float16))
        in_maps.append({"xt": xt, "wc": wc_dev, "wf": wf_dev})
    return in_maps


_NC_CACHE = {}


def _get_nc():
    if "nc" not in _NC_CACHE:
        _NC_CACHE["nc"] = build_bass()
    return _NC_CACHE["nc"]


def _run(x, W_x, b_x, W_fx, b_fx, W_slice, b_slice, temperature, trace=False):
    wc_dev, wf_dev = make_device_weights(W_x, b_x, W_fx, W_slice, b_slice, temperature)
    in_maps = make_in_maps(x, wc_dev, wf_dev)
    res = run_bass_kernel_spmd(_get_nc(), in_maps, core_ids=list(range(NCORES)),
                               trace=trace)
    out = postprocess([r["out"] for r in res.results], b_fx)
    return out, res


def kernel(**inputs) -> np.ndarray:
    out, _ = _run(**inputs)
    return out


def kernel_traced(**inputs):
    out, res = _run(**inputs, trace=True)
    return out, res
